# revision 3
# baseline (speedup 1.0000x reference)
import math
from contextlib import ExitStack

import numpy as np

N, T, D, H = 512, 128, 512, 512
NC = 8
n = N // NC          # 64 samples per core
H4 = 4 * H           # 2048
SCALE = 1.0 / math.sqrt(H)
DEBUG_TAPS = True

_cache = {}


def _build_kernel():
    if "nc" in _cache:
        return _cache["nc"]

    import concourse.bass as bass
    import concourse.tile as tile
    from concourse import bacc, mybir

    f32 = mybir.dt.float32
    bf16 = mybir.dt.bfloat16
    ALU = mybir.AluOpType
    ACTF = mybir.ActivationFunctionType
    AX = mybir.AxisListType

    nc = bacc.Bacc(
        "TRN2",
        target_bir_lowering=False,
        debug=False,
        enable_asserts=False,
        num_devices=NC,
    )

    xT = nc.dram_tensor("xT", (D, n * T), bf16, kind="ExternalInput").ap()
    A_sm = nc.dram_tensor("A_sm", (n, 16 * H), f32, kind="ExternalInput").ap()
    Wc = nc.dram_tensor("Wc", (128, 12 * H4), bf16, kind="ExternalInput").ap()
    bvec = nc.dram_tensor("bvec", (1, H4), bf16, kind="ExternalInput").ap()
    ident = nc.dram_tensor("ident", (n, n), f32, kind="ExternalInput").ap()
    onesv = nc.dram_tensor("onesv", (1, n), bf16, kind="ExternalInput").ap()
    hs = nc.dram_tensor("hs", (T, n, H), f32, kind="ExternalOutput").ap()
    dbg = None
    if DEBUG_TAPS:
        dbg = {nm: nc.dram_tensor(f"dbg_{nm}", shp, f32, kind="ExternalOutput").ap()
               for nm, shp in [("h0", (n, H)), ("dot", (n, 16)), ("wexp", (n, 16)),
                               ("attn", (n, H)), ("hb", (n, H4)), ("hT", (128, 4 * n)),
                               ("xts", (128, 4 * n))]}

    with tile.TileContext(nc) as tc, ExitStack() as ctx:
        const_pool = ctx.enter_context(tc.tile_pool(name="const", bufs=1))
        xts_pool = ctx.enter_context(tc.tile_pool(name="xts", bufs=3))
        attnT_pool = ctx.enter_context(tc.tile_pool(name="attnT", bufs=2))
        work = ctx.enter_context(tc.tile_pool(name="work", bufs=1))
        psum_mm = ctx.enter_context(tc.tile_pool(name="psum_mm", bufs=1, space="PSUM"))
        psum_tp = ctx.enter_context(tc.tile_pool(name="psum_tp", bufs=4, space="PSUM"))

        # ---- persistent tiles -------------------------------------------------
        W_sb = const_pool.tile([128, 12 * H4], bf16)  # chunk c cols [H4*c, H4*(c+1))
        nc.sync.dma_start(W_sb[:], Wc[:])
        b_sb = const_pool.tile([1, H4], bf16)
        nc.sync.dma_start(b_sb[:], bvec[:])
        id_sb = const_pool.tile([n, n], f32)
        nc.sync.dma_start(id_sb[:], ident[:])
        A_sb = const_pool.tile([n, 16 * H], f32)      # row layout p*H + h
        nc.sync.dma_start(A_sb[:], A_sm[:])

        hT = const_pool.tile([128, 4 * n], bf16)      # h^T: feature 128*ci+part
        c_st = const_pool.tile([n, H], f32)
        h_sm = const_pool.tile([n, H], f32)
        ones_row = const_pool.tile([1, n], bf16)
        nc.sync.dma_start(ones_row[:], onesv[:])

        # ---- h0 = mean over p of A_flat --------------------------------------
        h0_tmp = const_pool.tile([n, H], f32)
        A_grp = A_sb[:].rearrange("s (p h) -> s h p", p=16)
        nc.vector.tensor_reduce(h0_tmp[:], A_grp, axis=AX.X, op=ALU.add)
        nc.scalar.activation(h_sm[:], h0_tmp[:], ACTF.Copy, scale=1.0 / 16.0)
        nc.scalar.activation(c_st[:], h0_tmp[:], ACTF.Copy, scale=1.0 / 16.0)
        for ci in range(4):
            pt = psum_tp.tile([128, n], f32)
            nc.tensor.transpose(pt[:], h_sm[:, 128 * ci:128 * (ci + 1)], id_sb[:])
            nc.vector.tensor_copy(hT[:, n * ci:n * (ci + 1)], pt[:])

        if DEBUG_TAPS:
            nc.sync.dma_start(dbg["h0"], h_sm[:])
        xT_r = xT.rearrange("(c p) (nn tt) -> tt c p nn", p=128, tt=T)

        # ---- recurrence -------------------------------------------------------
        for t in range(T):
            # prefetch x_t^T   [512 feat, 64 samples] as 4 chunks of [128, 64]
            xts = xts_pool.tile([128, 4 * n], bf16)
            for ci in range(4):
                nc.sync.dma_start(xts[:, n * ci:n * (ci + 1)], xT_r[t, ci])

            # ---- attention ----------------------------------------------------
            # dot[s, p] = sum_h A[s, p, h] * h[s, h]
            prod = work.tile([n, 16 * H], f32)
            h_b = h_sm[:].rearrange("s (r h) -> s r h", r=1).broadcast_to([n, 16, H])
            nc.vector.tensor_tensor(
                prod[:].rearrange("s (p h) -> s p h", p=16),
                A_sb[:].rearrange("s (p h) -> s p h", p=16),
                h_b, ALU.mult)
            dot = work.tile([n, 16], f32)
            nc.vector.tensor_reduce(
                dot[:], prod[:].rearrange("s (p h) -> s p h", p=16),
                axis=AX.X, op=ALU.add)

            if DEBUG_TAPS and t == 0:
                nc.sync.dma_start(dbg["dot"], dot[:])
            # softmax over p (scale folded into exp)
            m = work.tile([n, 1], f32)
            mb = work.tile([n, 1], f32)
            wexp = work.tile([n, 16], f32)
            ssum = work.tile([n, 1], f32)
            rinv = work.tile([n, 1], f32)
            nc.vector.tensor_reduce(m[:], dot[:], axis=AX.X, op=ALU.max)
            nc.scalar.activation(mb[:], m[:], ACTF.Copy, scale=-SCALE)
            nc.scalar.activation(wexp[:], dot[:], ACTF.Exp, bias=mb[:], scale=SCALE)
            nc.vector.tensor_reduce(ssum[:], wexp[:], axis=AX.X, op=ALU.add)
            nc.vector.reciprocal(rinv[:], ssum[:])

            if DEBUG_TAPS and t == 0:
                nc.sync.dma_start(dbg["wexp"], wexp[:])
            # attn[s, h] = (sum_p wexp[s, p] * A[s, p, h]) * rinv[s]
            prodA = work.tile([n, 16 * H], f32)
            w_b = wexp[:].rearrange("s (p r) -> s p r", r=1).broadcast_to([n, 16, H])
            nc.vector.tensor_tensor(
                prodA[:].rearrange("s (p h) -> s p h", p=16),
                A_sb[:].rearrange("s (p h) -> s p h", p=16),
                w_b, ALU.mult)
            attn_u = work.tile([n, H], f32)
            nc.vector.tensor_reduce(
                attn_u[:], prodA[:].rearrange("s (p h) -> s h p", p=16),
                axis=AX.X, op=ALU.add)
            attn_sm = work.tile([n, H], f32)
            r_b = rinv[:].broadcast_to([n, H])
            nc.vector.tensor_tensor(attn_sm[:], attn_u[:], r_b, ALU.mult)

            if DEBUG_TAPS and t == 0:
                nc.sync.dma_start(dbg["attn"], attn_sm[:])
            attnT = attnT_pool.tile([128, 4 * n], bf16)
            for ci in range(4):
                pt = psum_tp.tile([128, n], f32)
                nc.tensor.transpose(pt[:], attn_sm[:, 128 * ci:128 * (ci + 1)],
                                    id_sb[:])
                nc.vector.tensor_copy(attnT[:, n * ci:n * (ci + 1)], pt[:])

            # ---- big matmul: hbar = h@Wh + attn@Wattn + x_t@Wx + b ------------
            hb = psum_mm.tile([n, H4], f32)
            for j in range(4):
                cols = slice(512 * j, 512 * (j + 1))
                for c in [0, 1, 2, 3, 8, 9, 10, 11, 12, 4, 5, 6, 7]:
                    if c < 4:          # h chunks
                        lhsT = hT[:, n * c:n * (c + 1)]
                        rhs = W_sb[:, H4 * c + 512 * j:H4 * c + 512 * (j + 1)]
                    elif c < 8:        # attn chunks
                        lhsT = attnT[:, n * (c - 4):n * (c - 3)]
                        rhs = W_sb[:, H4 * c + 512 * j:H4 * c + 512 * (j + 1)]
                    elif c < 12:       # x chunks
                        lhsT = xts[:, n * (c - 8):n * (c - 7)]
                        rhs = W_sb[:, H4 * c + 512 * j:H4 * c + 512 * (j + 1)]
                    else:              # bias row
                        lhsT = ones_row[:]
                        rhs = b_sb[:, cols]
                    nc.tensor.matmul(
                        hb[:, cols],
                        lhsT,
                        rhs,
                        start=(c == 0),
                        stop=(c == 7),
                    )

            if DEBUG_TAPS and t == 0:
                hbc = work.tile([n, H4], f32)
                nc.vector.tensor_copy(hbc[:], hb[:])
                nc.sync.dma_start(dbg["hb"], hbc[:])
            # ---- gates --------------------------------------------------------
            gifo = work.tile([n, 3 * H], f32)
            gg = work.tile([n, H], f32)
            nc.scalar.activation(gifo[:], hb[:, 0:3 * H], ACTF.Sigmoid)
            nc.scalar.activation(gg[:], hb[:, 3 * H:4 * H], ACTF.Tanh)

            t1 = work.tile([n, H], f32)
            t2 = work.tile([n, H], f32)
            nc.vector.tensor_mul(t1[:], gifo[:, H:2 * H], c_st[:])
            nc.vector.tensor_mul(t2[:], gifo[:, 0:H], gg[:])
            nc.vector.tensor_add(c_st[:], t1[:], t2[:])
            tct = work.tile([n, H], f32)
            nc.scalar.activation(tct[:], c_st[:], ACTF.Tanh)
            nc.vector.tensor_mul(h_sm[:], gifo[:, 2 * H:3 * H], tct[:])

            nc.sync.dma_start(hs[t], h_sm[:])

            for ci in range(4):
                pt = psum_tp.tile([128, n], f32)
                nc.tensor.transpose(pt[:], h_sm[:, 128 * ci:128 * (ci + 1)],
                                    id_sb[:])
                nc.vector.tensor_copy(hT[:, n * ci:n * (ci + 1)], pt[:])

        if DEBUG_TAPS:
            nc.gpsimd.dma_start(dbg["hT"], hT[:])
    nc.compile()
    _cache["nc"] = nc
    return nc


LAST_RESULT = None


def kernel(x, A, Wx, Wh, Wattn, b):
    import os
    import ml_dtypes
    from concourse import bass_utils

    nc = _build_kernel()
    bft = ml_dtypes.bfloat16

    Wcat = np.concatenate([np.asarray(Wh), np.asarray(Wattn), np.asarray(Wx)],
                          axis=0)                         # (1536, 2048)
    Wc_host = np.ascontiguousarray(
        Wcat.reshape(12, 128, H4).transpose(1, 0, 2).reshape(128, 12 * H4)
    ).astype(bft)
    b_host = np.asarray(b, dtype=np.float32).reshape(1, H4).astype(bft)
    ident = np.eye(n, dtype=np.float32)
    ones_h = np.ones((1, n), dtype=bft)

    in_maps = []
    for k in range(NC):
        xc = np.asarray(x[n * k:n * (k + 1)], dtype=np.float32)   # (64, T, D)
        Ac = np.asarray(A[n * k:n * (k + 1)], dtype=np.float32)   # (64, H, 4, 4)
        xT_host = np.ascontiguousarray(
            xc.transpose(2, 0, 1).reshape(D, n * T)).astype(bft)
        A_host = np.ascontiguousarray(
            Ac.reshape(n, H, 16).transpose(0, 2, 1).reshape(n, 16 * H))
        in_maps.append({
            "xT": xT_host,
            "A_sm": A_host,
            "Wc": Wc_host,
            "bvec": b_host,
            "ident": ident,
            "onesv": ones_h,
        })

    trace = os.environ.get("KERNEL_TRACE") == "1"
    tmpdir = os.environ.get("KERNEL_TRACE_DIR") or None
    res = bass_utils.run_bass_kernel_spmd(
        nc, in_maps, core_ids=list(range(NC)), trace=trace, tmpdir=tmpdir
    )
    global LAST_RESULT
    LAST_RESULT = res

    out = np.empty((N, T, H), dtype=np.float32)
    for k in range(NC):
        hs_k = np.asarray(res.results[k]["hs"])           # (T, n, H)
        out[n * k:n * (k + 1)] = hs_k.transpose(1, 0, 2)
    return out



# revision 13
# speedup vs baseline: 1.7191x; 1.7191x over previous
import math
from contextlib import ExitStack

import numpy as np

N, T, D, H = 512, 128, 512, 512
NC = 8
n = N // NC          # 64 samples per core
H4 = 4 * H           # 2048
SCALE = 1.0 / math.sqrt(H)

_cache = {}


def _build_kernel():
    if "nc" in _cache:
        return _cache["nc"]

    import concourse.bass as bass
    import concourse.tile as tile
    from concourse import bacc, mybir

    f32 = mybir.dt.float32
    bf16 = mybir.dt.bfloat16
    ALU = mybir.AluOpType
    ACTF = mybir.ActivationFunctionType
    AX = mybir.AxisListType

    nc = bacc.Bacc(
        "TRN2",
        target_bir_lowering=False,
        debug=False,
        enable_asserts=False,
        num_devices=NC,
    )

    # host-preprocessed layouts (see kernel() below)
    xh = nc.dram_tensor("xh", (T, 128, 256), bf16, kind="ExternalInput").ap()
    A2d = nc.dram_tensor("A2d", (128, 8 * H), bf16, kind="ExternalInput").ap()
    A2bd = nc.dram_tensor("A2bd", (128, H * 8), bf16, kind="ExternalInput").ap()
    Wc = nc.dram_tensor("Wc", (128, 12 * H4), bf16, kind="ExternalInput").ap()
    bvec = nc.dram_tensor("bvec", (1, H4), bf16, kind="ExternalInput").ap()
    identd = nc.dram_tensor("identd", (n, n), bf16, kind="ExternalInput").ap()
    onesd = nc.dram_tensor("onesd", (1, n), bf16, kind="ExternalInput").ap()
    hs = nc.dram_tensor("hs", (T, n, H), bf16, kind="ExternalOutput").ap()

    with tile.TileContext(nc) as tc, ExitStack() as ctx:
        const_pool = ctx.enter_context(tc.tile_pool(name="const", bufs=1))
        xts_pool = ctx.enter_context(tc.tile_pool(name="xts", bufs=3))
        psum_mm = ctx.enter_context(tc.tile_pool(name="psum_mm", bufs=1, space="PSUM"))
        psum_tp = ctx.enter_context(tc.tile_pool(name="psum_tp", bufs=4, space="PSUM"))

        # ---- persistent tiles ------------------------------------------------
        W_sb = const_pool.tile([128, 12 * H4], bf16)
        b_sb = const_pool.tile([1, H4], bf16)
        id_sb = const_pool.tile([n, n], bf16)
        ones_row = const_pool.tile([1, n], bf16)
        A2 = const_pool.tile([128, 8 * H], bf16)     # q=(pb,s); free=(pl, h)
        A2b = const_pool.tile([128, H * 8], bf16)    # q=(pb,s); free=(h, pl)
        nc.sync.dma_start(W_sb[:], Wc[:])
        nc.sync.dma_start(b_sb[:], bvec[:])
        nc.sync.dma_start(id_sb[:], identd[:])
        nc.sync.dma_start(ones_row[:], onesd[:])
        nc.sync.dma_start(A2[:], A2d[:])
        nc.sync.dma_start(A2b[:], A2bd[:])

        h2 = const_pool.tile([128, H], bf16)         # h duplicated on both halves
        hT = const_pool.tile([128, 4 * n], bf16)     # h^T: chunk ci at cols 64ci
        attnT = const_pool.tile([128, 4 * n], bf16)
        c_st = const_pool.tile([n, H], f32)

        # per-step scratch (persistent; deps handled by tile framework)
        prodD = const_pool.tile([128, 8 * H], bf16)
        dot_sb = const_pool.tile([128, 8], bf16)
        th = const_pool.tile([128, 8], f32)
        thp1 = const_pool.tile([128, 8], bf16)
        om = const_pool.tile([128, 8], f32)
        rin = const_pool.tile([128, 8], f32)
        wexp = const_pool.tile([128, 8], bf16)
        s8 = const_pool.tile([128, 1], f32)
        s8hi_c = const_pool.tile([n, 1], f32)
        ssum = const_pool.tile([n, 1], f32)
        rs = const_pool.tile([n, 1], f32)
        prodA = const_pool.tile([128, H * 8], bf16)
        attn2 = const_pool.tile([128, H], bf16)
        attn2hi_c = const_pool.tile([n, H], bf16)
        attn_ps = const_pool.tile([n, H], bf16)
        attn_sm = const_pool.tile([n, H], bf16)
        gi = const_pool.tile([n, H], bf16)
        gf = const_pool.tile([n, H], bf16)
        go = const_pool.tile([n, H], bf16)
        gg = const_pool.tile([n, H], bf16)
        t1 = const_pool.tile([n, H], f32)
        t2 = const_pool.tile([n, H], f32)
        tct = const_pool.tile([n, H], bf16)
        h0p = const_pool.tile([128, H], f32)
        h0phi_c = const_pool.tile([n, H], f32)
        h0h = const_pool.tile([n, H], f32)

        # ---- h0 = mean over p of A_flat; c0 = h0 -----------------------------
        nc.vector.tensor_reduce(
            h0p[:], A2b[:].rearrange("q (h pl) -> q h pl", pl=8),
            axis=AX.X, op=ALU.add)
        nc.vector.tensor_copy(h0phi_c[:], h0p[n:128, :])
        nc.vector.tensor_tensor(h0h[:], h0p[0:n, :], h0phi_c[:], ALU.add)
        nc.scalar.activation(c_st[:], h0h[:], ACTF.Copy, scale=1.0 / 16.0)
        nc.scalar.activation(h2[0:n, :], h0h[:], ACTF.Copy, scale=1.0 / 16.0)
        nc.vector.tensor_copy(h2[n:128, :], h2[0:n, :])
        for ci in range(4):
            pt = psum_tp.tile([128, n], bf16)
            nc.tensor.transpose(pt[:], h2[0:n, 128 * ci:128 * (ci + 1)], id_sb[:])
            nc.scalar.copy(hT[:, n * ci:n * (ci + 1)], pt[:])

        # prefetch x for t=0,1
        xts_tiles = {}
        for tpre in range(2):
            xt0 = xts_pool.tile([128, 4 * n], bf16)
            nc.sync.dma_start(xt0[:], xh[tpre])
            xts_tiles[tpre] = xt0

        # ---- recurrence ------------------------------------------------------
        for t in range(T):
            if t + 2 < T:
                xtp = xts_pool.tile([128, 4 * n], bf16)
                nc.sync.dma_start(xtp[:], xh[t + 2])
                xts_tiles[t + 2] = xtp
            xts = xts_tiles.pop(t)

            # -- phase A matmuls (h, x, bias) — no attention dependency --------
            hb = psum_mm.tile([n, H4], f32)
            for j in range(4):
                cols = slice(512 * j, 512 * (j + 1))
                for c in [0, 1, 2, 3, 8, 9, 10, 11, 12]:
                    if c < 4:
                        lhsT = hT[:, n * c:n * (c + 1)]
                        rhs = W_sb[:, H4 * c + 512 * j:H4 * c + 512 * (j + 1)]
                    elif c < 12:
                        lhsT = xts[:, n * (c - 8):n * (c - 7)]
                        rhs = W_sb[:, H4 * c + 512 * j:H4 * c + 512 * (j + 1)]
                    else:
                        lhsT = ones_row[:]
                        rhs = b_sb[:, cols]
                    nc.tensor.matmul(hb[:, cols], lhsT, rhs,
                                     start=(c == 0), stop=False)

            # -- attention: dot[s,p] = sum_h A[s,p,h]*h[s,h]
            h_b = h2[:].rearrange("q (r h) -> q r h", r=1).broadcast_to(
                [128, 8, H])
            nc.vector.tensor_tensor(
                prodD[:].rearrange("q (pl h) -> q pl h", pl=8),
                A2[:].rearrange("q (pl h) -> q pl h", pl=8),
                h_b, ALU.mult)
            with nc.allow_low_precision(reason="bf16 reduce out, f32 internal"):
                nc.vector.tensor_reduce(
                    dot_sb[:], prodD[:].rearrange("q (pl h) -> q pl h", pl=8),
                    axis=AX.X, op=ALU.add)
            # softmax via exp(x) = (1+tanh(x/2))/(1-tanh(x/2)); norm folded below
            nc.scalar.activation(th[:], dot_sb[:], ACTF.Tanh, scale=0.5 * SCALE)
            nc.scalar.activation(thp1[:], th[:], ACTF.Copy, bias=1.0)
            nc.scalar.activation(om[:], th[:], ACTF.Copy, bias=1.0, scale=-1.0)
            nc.vector.reciprocal(rin[:], om[:])
            nc.vector.tensor_tensor(wexp[:], thp1[:], rin[:], ALU.mult)
            nc.vector.tensor_reduce(s8[:], wexp[:], axis=AX.X, op=ALU.add)
            nc.vector.tensor_copy(s8hi_c[:], s8[n:128, :])
            nc.vector.tensor_tensor(ssum[:], s8[0:n, :], s8hi_c[:], ALU.add)
            nc.vector.reciprocal(rs[:], ssum[:])
            # attn[s,h] = (sum_p wexp[s,p]*A[s,p,h]) / ssum[s]
            w_b = wexp[:].rearrange("q (r pl) -> q r pl", r=1).broadcast_to(
                [128, H, 8])
            nc.vector.tensor_tensor(
                prodA[:].rearrange("q (h pl) -> q h pl", pl=8),
                A2b[:].rearrange("q (h pl) -> q h pl", pl=8),
                w_b, ALU.mult)
            with nc.allow_low_precision(reason="bf16 reduce out, f32 internal"):
                nc.vector.tensor_reduce(
                    attn2[:], prodA[:].rearrange("q (h pl) -> q h pl", pl=8),
                    axis=AX.X, op=ALU.add)
            nc.vector.tensor_copy(attn2hi_c[:], attn2[n:128, :])
            nc.vector.tensor_tensor(attn_ps[:], attn2[0:n, :], attn2hi_c[:],
                                    ALU.add)
            nc.scalar.activation(attn_sm[:], attn_ps[:], ACTF.Copy, scale=rs[:])

            for ci in range(4):
                pt = psum_tp.tile([128, n], bf16)
                nc.tensor.transpose(pt[:], attn_sm[:, 128 * ci:128 * (ci + 1)],
                                    id_sb[:])
                nc.scalar.copy(attnT[:, n * ci:n * (ci + 1)], pt[:])

            # -- phase B matmuls (attn) + pipelined gates ----------------------
            for j in [1, 0, 3, 2]:
                cols = slice(512 * j, 512 * (j + 1))
                for c in [4, 5, 6, 7]:
                    lhsT = attnT[:, n * (c - 4):n * (c - 3)]
                    rhs = W_sb[:, H4 * c + 512 * j:H4 * c + 512 * (j + 1)]
                    nc.tensor.matmul(hb[:, cols], lhsT, rhs,
                                     start=False, stop=(c == 7))
                if j == 1:
                    nc.scalar.activation(gf[:], hb[:, 512:1024], ACTF.Sigmoid)
                    nc.vector.tensor_tensor(t1[:], gf[:], c_st[:], ALU.mult)
                elif j == 0:
                    nc.scalar.activation(gi[:], hb[:, 0:512], ACTF.Sigmoid)
                elif j == 3:
                    nc.scalar.activation(gg[:], hb[:, 1536:2048], ACTF.Tanh)
                    nc.vector.tensor_tensor(t2[:], gi[:], gg[:], ALU.mult)
                    nc.vector.tensor_tensor(c_st[:], t1[:], t2[:], ALU.add)
                    nc.scalar.activation(tct[:], c_st[:], ACTF.Tanh)
                else:
                    nc.scalar.activation(go[:], hb[:, 1024:1536], ACTF.Sigmoid)
                    nc.vector.tensor_tensor(h2[0:n, :], go[:], tct[:], ALU.mult)
                    nc.vector.tensor_copy(h2[n:128, :], h2[0:n, :])

            nc.gpsimd.dma_start(hs[t], h2[0:n, :])

            for ci in range(4):
                pt = psum_tp.tile([128, n], bf16)
                nc.tensor.transpose(pt[:], h2[0:n, 128 * ci:128 * (ci + 1)],
                                    id_sb[:])
                nc.scalar.copy(hT[:, n * ci:n * (ci + 1)], pt[:])

    nc.compile()
    _cache["nc"] = nc
    return nc


LAST_RESULT = None


def kernel(x, A, Wx, Wh, Wattn, b):
    import os
    import ml_dtypes
    from concourse import bass_utils

    nc = _build_kernel()
    bft = ml_dtypes.bfloat16

    Wcat = np.concatenate([np.asarray(Wh), np.asarray(Wattn), np.asarray(Wx)],
                          axis=0)                         # (1536, 2048)
    Wc_host = np.ascontiguousarray(
        Wcat.reshape(12, 128, H4).transpose(1, 0, 2).reshape(128, 12 * H4)
    ).astype(bft)
    b_host = np.asarray(b, dtype=np.float32).reshape(1, H4).astype(bft)
    ident = np.eye(n, dtype=np.float32).astype(bft)
    ones_h = np.ones((1, n), dtype=bft)

    in_maps = []
    for k in range(NC):
        xc = np.asarray(x[n * k:n * (k + 1)], dtype=np.float32)   # (64, T, D)
        Ac = np.asarray(A[n * k:n * (k + 1)], dtype=np.float32)   # (64, H, 4, 4)
        xh_host = np.ascontiguousarray(
            xc.transpose(1, 2, 0).reshape(T, 4, 128, n)
            .transpose(0, 2, 1, 3).reshape(T, 128, 4 * n)).astype(bft)
        A_flat = Ac.reshape(n, H, 16).transpose(0, 2, 1)          # (n, 16, H)
        A4 = A_flat.reshape(n, 2, 8, H)
        A2_host = np.ascontiguousarray(
            A4.transpose(1, 0, 2, 3).reshape(128, 8 * H)).astype(bft)
        A2b_host = np.ascontiguousarray(
            A4.transpose(1, 0, 3, 2).reshape(128, H * 8)).astype(bft)
        in_maps.append({
            "xh": xh_host,
            "A2d": A2_host,
            "A2bd": A2b_host,
            "Wc": Wc_host,
            "bvec": b_host,
            "identd": ident,
            "onesd": ones_h,
        })

    trace = os.environ.get("KERNEL_TRACE") == "1"
    tmpdir = os.environ.get("KERNEL_TRACE_DIR") or None
    res = bass_utils.run_bass_kernel_spmd(
        nc, in_maps, core_ids=list(range(NC)), trace=trace, tmpdir=tmpdir
    )
    global LAST_RESULT
    LAST_RESULT = res

    out = np.empty((N, T, H), dtype=np.float32)
    for k in range(NC):
        hs_k = np.asarray(res.results[k]["hs"])           # (T, n, H) bf16
        out[n * k:n * (k + 1)] = hs_k.transpose(1, 0, 2).astype(np.float32)
    return out


# revision 16
# speedup vs baseline: 2.1705x; 1.2625x over previous
import math
from contextlib import ExitStack

import numpy as np

N, T, D, H = 512, 128, 512, 512
NC = 8
n = N // NC          # 64 samples per core
H4 = 4 * H           # 2048
SCALE = 1.0 / math.sqrt(H)

_cache = {}


def _build_kernel():
    if "nc" in _cache:
        return _cache["nc"]

    import concourse.bass as bass
    import concourse.tile as tile
    from concourse import bacc, mybir

    f32 = mybir.dt.float32
    bf16 = mybir.dt.bfloat16
    ALU = mybir.AluOpType
    ACTF = mybir.ActivationFunctionType
    AX = mybir.AxisListType

    nc = bacc.Bacc(
        "TRN2",
        target_bir_lowering=False,
        debug=False,
        enable_asserts=False,
        num_devices=NC,
    )

    # host-preprocessed layouts (see kernel() below)
    xh = nc.dram_tensor("xh", (T, 128, 256), bf16, kind="ExternalInput").ap()
    A2d = nc.dram_tensor("A2d", (128, 8 * H), bf16, kind="ExternalInput").ap()
    A2bd = nc.dram_tensor("A2bd", (128, H * 8), bf16, kind="ExternalInput").ap()
    Wc = nc.dram_tensor("Wc", (128, 12 * H4), bf16, kind="ExternalInput").ap()
    bvec = nc.dram_tensor("bvec", (1, H4), bf16, kind="ExternalInput").ap()
    identd = nc.dram_tensor("identd", (n, n), bf16, kind="ExternalInput").ap()
    onesd = nc.dram_tensor("onesd", (1, n), bf16, kind="ExternalInput").ap()
    hs = nc.dram_tensor("hs", (T, n, H), bf16, kind="ExternalOutput").ap()

    with tile.TileContext(nc) as tc, ExitStack() as ctx:
        const_pool = ctx.enter_context(tc.tile_pool(name="const", bufs=1))
        xts_pool = ctx.enter_context(tc.tile_pool(name="xts", bufs=3))
        psum_mm = ctx.enter_context(tc.tile_pool(name="psum_mm", bufs=2, space="PSUM"))
        psum_tp = ctx.enter_context(tc.tile_pool(name="psum_tp", bufs=4, space="PSUM"))

        # ---- persistent tiles ------------------------------------------------
        W_sb = const_pool.tile([128, 12 * H4], bf16)
        b_sb = const_pool.tile([1, H4], bf16)
        id_sb = const_pool.tile([n, n], bf16)
        ones_row = const_pool.tile([1, n], bf16)
        A2 = const_pool.tile([128, 8 * H], bf16)     # q=(pb,s); free=(pl, h)
        A2b = const_pool.tile([128, H * 8], bf16)    # q=(pb,s); free=(h, pl)
        nc.sync.dma_start(W_sb[:], Wc[:])
        nc.sync.dma_start(b_sb[:], bvec[:])
        nc.sync.dma_start(id_sb[:], identd[:])
        nc.sync.dma_start(ones_row[:], onesd[:])
        nc.sync.dma_start(A2[:], A2d[:])
        nc.sync.dma_start(A2b[:], A2bd[:])

        h2 = const_pool.tile([128, H], bf16)         # h duplicated on both halves
        hT = const_pool.tile([128, 4 * n], bf16)     # h^T: chunk ci at cols 64ci
        attnT = const_pool.tile([128, 4 * n], bf16)
        c_st = const_pool.tile([n, H], bf16)

        # per-step scratch (persistent; deps handled by tile framework)
        dotscr = const_pool.tile([128, H], bf16)
        dot_sb = const_pool.tile([128, 8], f32)
        th = const_pool.tile([128, 8], f32)
        thp1 = const_pool.tile([128, 8], bf16)
        om = const_pool.tile([128, 8], f32)
        rin = const_pool.tile([128, 8], f32)
        wexp = const_pool.tile([128, 8], bf16)
        s8 = const_pool.tile([128, 1], f32)
        s8hi_c = const_pool.tile([n, 1], f32)
        ssum = const_pool.tile([n, 1], f32)
        rs = const_pool.tile([n, 1], f32)
        prodA = const_pool.tile([128, H * 8], bf16)  # (h, pl) layout
        tr4 = const_pool.tile([128, H * 4], bf16)
        tr2 = const_pool.tile([128, H * 2], bf16)
        attn2 = const_pool.tile([128, H], bf16)
        attn2hi_c = const_pool.tile([n, H], bf16)
        attn_ps = const_pool.tile([n, H], bf16)
        attn_sm = const_pool.tile([n, H], bf16)
        gi = const_pool.tile([n, H], bf16)
        gf = const_pool.tile([n, H], bf16)
        go = const_pool.tile([n, H], bf16)
        gg = const_pool.tile([n, H], bf16)
        t1 = const_pool.tile([n, H], bf16)
        t2 = const_pool.tile([n, H], bf16)
        tct = const_pool.tile([n, H], bf16)
        h0p = const_pool.tile([128, H], f32)
        h0phi_c = const_pool.tile([n, H], f32)
        h0h = const_pool.tile([n, H], f32)

        # ---- h0 = mean over p of A_flat; c0 = h0 -----------------------------
        nc.vector.tensor_reduce(
            h0p[:], A2b[:].rearrange("q (h pl) -> q h pl", pl=8),
            axis=AX.X, op=ALU.add)
        nc.vector.tensor_copy(h0phi_c[:], h0p[n:128, :])
        nc.vector.tensor_tensor(h0h[:], h0p[0:n, :], h0phi_c[:], ALU.add)
        nc.scalar.activation(c_st[:], h0h[:], ACTF.Copy, scale=1.0 / 16.0)
        nc.scalar.activation(h2[0:n, :], h0h[:], ACTF.Copy, scale=1.0 / 16.0)
        nc.vector.tensor_copy(h2[n:128, :], h2[0:n, :])
        for ci in range(4):
            pt = psum_tp.tile([128, n], bf16)
            nc.tensor.transpose(pt[:], h2[0:n, 128 * ci:128 * (ci + 1)], id_sb[:])
            nc.scalar.copy(hT[:, n * ci:n * (ci + 1)], pt[:])

        # prefetch x for t=0,1
        xts_tiles = {}
        for tpre in range(2):
            xt0 = xts_pool.tile([128, 4 * n], bf16)
            nc.sync.dma_start(xt0[:], xh[tpre])
            xts_tiles[tpre] = xt0

        # col-tiled matmul layout: psum tile ps[128, 1024]
        #   ps[0:64,   0:512]  = hbar cols    0:512  (i)   j=0, tile (0,0)
        #   ps[64:128, 0:512]  = hbar cols  512:1024 (f)   j=1, tile (0,64)
        #   ps[0:64, 512:1024] = hbar cols 1024:1536 (o)   j=2, tile (0,0)
        #   ps[64:128,512:1024]= hbar cols 1536:2048 (g)   j=3, tile (0,64)
        def mm(ps, c, j, lhsT, start, stop):
            rhs = (b_sb[:, 512 * j:512 * (j + 1)] if c == 12 else
                   W_sb[:, H4 * c + 512 * j:H4 * c + 512 * (j + 1)])
            lo = (j % 2 == 0)
            out = (ps[0:n, 512 * (j // 2):512 * (j // 2 + 1)] if lo else
                   ps[n:128, 512 * (j // 2):512 * (j // 2 + 1)])
            nc.tensor.matmul(out, lhsT, rhs, start=start, stop=stop,
                             tile_position=(0, 0) if lo else (0, n),
                             skip_group_check=True)

        # ---- recurrence ------------------------------------------------------
        for t in range(T):
            if t + 2 < T:
                xtp = xts_pool.tile([128, 4 * n], bf16)
                nc.sync.dma_start(xtp[:], xh[t + 2])
                xts_tiles[t + 2] = xtp
            xts = xts_tiles.pop(t)

            # -- phase A matmuls (h, x, bias) — no attention dependency --------
            ps = psum_mm.tile([128, 2 * 512], f32)
            for c in [0, 1, 2, 3, 8, 9, 10, 11, 12]:
                if c < 4:
                    lhsT = hT[:, n * c:n * (c + 1)]
                elif c < 12:
                    lhsT = xts[:, n * (c - 8):n * (c - 7)]
                else:
                    lhsT = ones_row[:]
                for j in range(4):
                    mm(ps, c, j, lhsT, start=(c == 0), stop=False)

            # -- attention: dot[s,p] = sum_h A[s,p,h]*h[s,h] (fused stt) -------
            for pl in range(8):
                nc.vector.scalar_tensor_tensor(
                    out=dotscr[:], in0=A2[:, H * pl:H * (pl + 1)], scalar=SCALE,
                    in1=h2[:], op0=ALU.mult, op1=ALU.mult,
                    accum_out=dot_sb[:, pl:pl + 1])
            # softmax via exp(x) = (1+tanh(x/2))/(1-tanh(x/2)); norm folded below
            nc.scalar.activation(th[:], dot_sb[:], ACTF.Tanh, scale=0.5)
            nc.scalar.activation(thp1[:], th[:], ACTF.Copy, bias=1.0)
            nc.scalar.activation(om[:], th[:], ACTF.Copy, bias=1.0, scale=-1.0)
            nc.vector.reciprocal(rin[:], om[:])
            nc.vector.tensor_tensor(wexp[:], thp1[:], rin[:], ALU.mult)
            nc.vector.tensor_reduce(s8[:], wexp[:], axis=AX.X, op=ALU.add)
            nc.vector.tensor_copy(s8hi_c[:], s8[n:128, :])
            nc.vector.tensor_tensor(ssum[:], s8[0:n, :], s8hi_c[:], ALU.add)
            nc.vector.reciprocal(rs[:], ssum[:])
            # attn[s,h] = (sum_p wexp[s,p]*A[s,p,h]) / ssum[s]  (tree-add)
            w_b = wexp[:].rearrange("q (r pl) -> q r pl", r=1).broadcast_to(
                [128, H, 8])
            nc.vector.tensor_tensor(
                prodA[:].rearrange("q (h pl) -> q h pl", pl=8),
                A2b[:].rearrange("q (h pl) -> q h pl", pl=8),
                w_b, ALU.mult)
            pA = prodA[:].rearrange("q (h pl) -> q h pl", pl=8)
            v4 = tr4[:].rearrange("q (h pl) -> q h pl", pl=4)
            v2 = tr2[:].rearrange("q (h pl) -> q h pl", pl=2)
            nc.vector.tensor_tensor(v4, pA[:, :, 0:4], pA[:, :, 4:8], ALU.add)
            nc.vector.tensor_tensor(v2, v4[:, :, 0:2], v4[:, :, 2:4], ALU.add)
            nc.vector.tensor_tensor(
                attn2[:].rearrange("q (h r) -> q h r", r=1),
                v2[:, :, 0:1], v2[:, :, 1:2], ALU.add)
            nc.vector.tensor_copy(attn2hi_c[:], attn2[n:128, :])
            nc.vector.tensor_tensor(attn_ps[:], attn2[0:n, :], attn2hi_c[:],
                                    ALU.add)
            nc.scalar.activation(attn_sm[:], attn_ps[:], ACTF.Copy, scale=rs[:])

            for ci in range(4):
                pt = psum_tp.tile([128, n], bf16)
                nc.tensor.transpose(pt[:], attn_sm[:, 128 * ci:128 * (ci + 1)],
                                    id_sb[:])
                nc.scalar.copy(attnT[:, n * ci:n * (ci + 1)], pt[:])

            # -- phase B matmuls (attn) + pipelined gates ----------------------
            for c in [4, 5, 6, 7]:
                lhsT = attnT[:, n * (c - 4):n * (c - 3)]
                mm(ps, c, 0, lhsT, start=False, stop=(c == 7))
                mm(ps, c, 1, lhsT, start=False, stop=(c == 7))
            nc.scalar.activation(gi[:], ps[0:n, 0:512], ACTF.Sigmoid)
            nc.scalar.activation(gf[:], ps[n:128, 0:512], ACTF.Sigmoid)
            nc.vector.tensor_tensor(t1[:], gf[:], c_st[:], ALU.mult)
            for c in [4, 5, 6, 7]:
                lhsT = attnT[:, n * (c - 4):n * (c - 3)]
                mm(ps, c, 2, lhsT, start=False, stop=(c == 7))
                mm(ps, c, 3, lhsT, start=False, stop=(c == 7))
            nc.scalar.activation(gg[:], ps[n:128, 512:1024], ACTF.Tanh)
            nc.scalar.activation(go[:], ps[0:n, 512:1024], ACTF.Sigmoid)
            nc.vector.tensor_tensor(t2[:], gi[:], gg[:], ALU.mult)
            nc.vector.tensor_tensor(c_st[:], t1[:], t2[:], ALU.add)
            nc.scalar.activation(tct[:], c_st[:], ACTF.Tanh)
            nc.vector.tensor_tensor(h2[0:n, :], go[:], tct[:], ALU.mult)
            nc.vector.tensor_copy(h2[n:128, :], h2[0:n, :])

            nc.gpsimd.dma_start(hs[t], h2[0:n, :])

            for ci in range(4):
                pt = psum_tp.tile([128, n], bf16)
                nc.tensor.transpose(pt[:], h2[0:n, 128 * ci:128 * (ci + 1)],
                                    id_sb[:])
                nc.scalar.copy(hT[:, n * ci:n * (ci + 1)], pt[:])

    nc.compile()
    _cache["nc"] = nc
    return nc


LAST_RESULT = None


def kernel(x, A, Wx, Wh, Wattn, b):
    import os
    import ml_dtypes
    from concourse import bass_utils

    nc = _build_kernel()
    bft = ml_dtypes.bfloat16

    Wcat = np.concatenate([np.asarray(Wh), np.asarray(Wattn), np.asarray(Wx)],
                          axis=0)                         # (1536, 2048)
    Wc_host = np.ascontiguousarray(
        Wcat.reshape(12, 128, H4).transpose(1, 0, 2).reshape(128, 12 * H4)
    ).astype(bft)
    b_host = np.asarray(b, dtype=np.float32).reshape(1, H4).astype(bft)
    ident = np.eye(n, dtype=np.float32).astype(bft)
    ones_h = np.ones((1, n), dtype=bft)

    in_maps = []
    for k in range(NC):
        xc = np.asarray(x[n * k:n * (k + 1)], dtype=np.float32)   # (64, T, D)
        Ac = np.asarray(A[n * k:n * (k + 1)], dtype=np.float32)   # (64, H, 4, 4)
        xh_host = np.ascontiguousarray(
            xc.transpose(1, 2, 0).reshape(T, 4, 128, n)
            .transpose(0, 2, 1, 3).reshape(T, 128, 4 * n)).astype(bft)
        A_flat = Ac.reshape(n, H, 16).transpose(0, 2, 1)          # (n, 16, H)
        A4 = A_flat.reshape(n, 2, 8, H)
        A2_host = np.ascontiguousarray(
            A4.transpose(1, 0, 2, 3).reshape(128, 8 * H)).astype(bft)
        A2b_host = np.ascontiguousarray(
            A4.transpose(1, 0, 3, 2).reshape(128, H * 8)).astype(bft)
        in_maps.append({
            "xh": xh_host,
            "A2d": A2_host,
            "A2bd": A2b_host,
            "Wc": Wc_host,
            "bvec": b_host,
            "identd": ident,
            "onesd": ones_h,
        })

    trace = os.environ.get("KERNEL_TRACE") == "1"
    tmpdir = os.environ.get("KERNEL_TRACE_DIR") or None
    res = bass_utils.run_bass_kernel_spmd(
        nc, in_maps, core_ids=list(range(NC)), trace=trace, tmpdir=tmpdir
    )
    global LAST_RESULT
    LAST_RESULT = res

    out = np.empty((N, T, H), dtype=np.float32)
    for k in range(NC):
        hs_k = np.asarray(res.results[k]["hs"])           # (T, n, H) bf16
        out[n * k:n * (k + 1)] = hs_k.transpose(1, 0, 2).astype(np.float32)
    return out


# revision 21
# speedup vs baseline: 2.5864x; 1.1916x over previous
import math
from contextlib import ExitStack

import numpy as np

N, T, D, H = 512, 128, 512, 512
NC = 8
n = N // NC          # 64 samples per core
H4 = 4 * H           # 2048
SCALE = 1.0 / math.sqrt(H)

_cache = {}


def _build_kernel():
    if "nc" in _cache:
        return _cache["nc"]

    import concourse.bass as bass
    import concourse.tile as tile
    from concourse import bacc, mybir

    f32 = mybir.dt.float32
    bf16 = mybir.dt.bfloat16
    ALU = mybir.AluOpType
    ACTF = mybir.ActivationFunctionType
    AX = mybir.AxisListType

    nc = bacc.Bacc(
        "TRN2",
        target_bir_lowering=False,
        debug=False,
        enable_asserts=False,
        num_devices=NC,
    )

    # host-preprocessed layouts (see kernel() below)
    xh = nc.dram_tensor("xh", (T, 128, 256), bf16, kind="ExternalInput").ap()
    A2d = nc.dram_tensor("A2d", (128, 8 * H), bf16, kind="ExternalInput").ap()
    A2bd = nc.dram_tensor("A2bd", (128, H * 8), bf16, kind="ExternalInput").ap()
    Wc = nc.dram_tensor("Wc", (128, 12 * H4), bf16, kind="ExternalInput").ap()
    bvec = nc.dram_tensor("bvec", (1, H4), bf16, kind="ExternalInput").ap()
    identd = nc.dram_tensor("identd", (n, n), bf16, kind="ExternalInput").ap()
    onesd = nc.dram_tensor("onesd", (1, n), bf16, kind="ExternalInput").ap()
    hs = nc.dram_tensor("hs", (T, n, H), bf16, kind="ExternalOutput").ap()

    with tile.TileContext(nc) as tc, ExitStack() as ctx:
        const_pool = ctx.enter_context(tc.tile_pool(name="const", bufs=1))
        xts_pool = ctx.enter_context(tc.tile_pool(name="xts", bufs=3))
        psum_mm = ctx.enter_context(tc.tile_pool(name="psum_mm", bufs=2, space="PSUM"))
        psum_tp = ctx.enter_context(tc.tile_pool(name="psum_tp", bufs=4, space="PSUM"))

        # ---- persistent tiles ------------------------------------------------
        W_sb = const_pool.tile([128, 12 * H4], bf16)
        b_sb = const_pool.tile([1, H4], bf16)
        id_sb = const_pool.tile([n, n], bf16)
        ones_row = const_pool.tile([1, n], bf16)
        A2 = const_pool.tile([128, 8 * H], bf16)     # q=(pb,s); free=(pl, h)
        A2b = const_pool.tile([128, H * 8], bf16)    # q=(pb,s); free=(h, pl)
        nc.sync.dma_start(W_sb[:], Wc[:])
        nc.sync.dma_start(b_sb[:], bvec[:])
        nc.sync.dma_start(id_sb[:], identd[:])
        nc.sync.dma_start(ones_row[:], onesd[:])
        nc.sync.dma_start(A2[:], A2d[:])
        nc.sync.dma_start(A2b[:], A2bd[:])

        h2 = const_pool.tile([128, H], bf16)         # h duplicated on both halves
        hT = const_pool.tile([128, 4 * n], bf16)     # h^T: chunk ci at cols 64ci
        attnT = const_pool.tile([128, 4 * n], bf16)
        c_st = const_pool.tile([n, H], bf16)

        # per-step scratch (persistent; deps handled by tile framework)
        dotscr = const_pool.tile([128, H], bf16)
        dot_sb = const_pool.tile([128, 8], f32)
        th = const_pool.tile([128, 8], f32)
        thp1 = const_pool.tile([128, 8], bf16)
        om = const_pool.tile([128, 8], f32)
        rin = const_pool.tile([128, 8], f32)
        wexp = const_pool.tile([128, 8], bf16)
        s8 = const_pool.tile([128, 1], f32)
        s8hi_c = const_pool.tile([n, 1], f32)
        ssum = const_pool.tile([n, 1], f32)
        rs = const_pool.tile([n, 1], f32)
        prodA = const_pool.tile([128, H * 8], bf16)  # (h, pl) layout
        tr4 = const_pool.tile([128, H * 4], bf16)
        tr2 = const_pool.tile([128, H * 2], bf16)
        attn2 = const_pool.tile([128, H], bf16)
        attn2hi_c = const_pool.tile([n, H], bf16)
        attn_ps = const_pool.tile([n, H], bf16)
        attn_sm = const_pool.tile([n, H], bf16)
        gi = const_pool.tile([n, H], bf16)
        gf = const_pool.tile([n, H], bf16)
        go = const_pool.tile([n, H], bf16)
        gg = const_pool.tile([n, H], bf16)
        t1 = const_pool.tile([n, H], bf16)
        t2 = const_pool.tile([n, H], bf16)
        tct = const_pool.tile([n, H], bf16)
        h0p = const_pool.tile([128, H], f32)
        h0phi_c = const_pool.tile([n, H], f32)
        h0h = const_pool.tile([n, H], f32)

        # ---- h0 = mean over p of A_flat; c0 = h0 -----------------------------
        nc.vector.tensor_reduce(
            h0p[:], A2b[:].rearrange("q (h pl) -> q h pl", pl=8),
            axis=AX.X, op=ALU.add)
        nc.vector.tensor_copy(h0phi_c[:], h0p[n:128, :])
        nc.vector.tensor_tensor(h0h[:], h0p[0:n, :], h0phi_c[:], ALU.add)
        nc.scalar.activation(c_st[:], h0h[:], ACTF.Copy, scale=1.0 / 16.0)
        nc.scalar.activation(h2[0:n, :], h0h[:], ACTF.Copy, scale=1.0 / 16.0)
        nc.vector.tensor_copy(h2[n:128, :], h2[0:n, :])
        for ci in range(4):
            pt = psum_tp.tile([128, n], bf16)
            nc.tensor.transpose(pt[:], h2[0:n, 128 * ci:128 * (ci + 1)], id_sb[:])
            nc.scalar.copy(hT[:, n * ci:n * (ci + 1)], pt[:])

        # prefetch x for t=0,1
        xts_tiles = {}
        for tpre in range(2):
            xt0 = xts_pool.tile([128, 4 * n], bf16)
            nc.sync.dma_start(xt0[:], xh[tpre])
            xts_tiles[tpre] = xt0
        ps_tiles = {}

        # col-tiled matmul layout: psum tile ps[128, 1024]
        #   ps[0:64,   0:512]  = hbar cols    0:512  (i)   j=0, tile (0,0)
        #   ps[64:128, 0:512]  = hbar cols  512:1024 (f)   j=1, tile (0,64)
        #   ps[0:64, 512:1024] = hbar cols 1024:1536 (o)   j=2, tile (0,0)
        #   ps[64:128,512:1024]= hbar cols 1536:2048 (g)   j=3, tile (0,64)
        def mm(ps, c, j, lhsT, start, stop):
            rhs = (b_sb[:, 512 * j:512 * (j + 1)] if c == 12 else
                   W_sb[:, H4 * c + 512 * j:H4 * c + 512 * (j + 1)])
            lo = (j % 2 == 0)
            out = (ps[0:n, 512 * (j // 2):512 * (j // 2 + 1)] if lo else
                   ps[n:128, 512 * (j // 2):512 * (j // 2 + 1)])
            nc.tensor.matmul(out, lhsT, rhs, start=start, stop=stop,
                             tile_position=(0, 0) if lo else (0, n),
                             skip_group_check=True)

        # x-part of step 0 opens the accumulation groups for ps_tiles[0]
        ps0 = psum_mm.tile([128, 2 * 512], f32, tag="psmm")
        xts0 = xts_tiles.pop(0)
        for c in [8, 9, 10, 11]:
            lhsT0 = xts0[:, n * (c - 8):n * (c - 7)]
            for j in range(4):
                mm(ps0, c, j, lhsT0, start=(c == 8), stop=False)
        ps_tiles[0] = ps0

        # ---- recurrence ------------------------------------------------------
        for t in range(T):
            if t + 2 < T:
                xtp = xts_pool.tile([128, 4 * n], bf16)
                nc.sync.dma_start(xtp[:], xh[t + 2])
                xts_tiles[t + 2] = xtp
            ps = ps_tiles.pop(t)

            # -- phase A matmuls (h, bias); x was done during step t-1 ---------
            for c in [0, 1, 2, 3, 12]:
                lhsT = hT[:, n * c:n * (c + 1)] if c < 4 else ones_row[:]
                for j in range(4):
                    mm(ps, c, j, lhsT, start=False, stop=False)

            # -- attention: dot[s,p] = sum_h A[s,p,h]*h[s,h] (fused stt) -------
            for pl in range(8):
                nc.vector.scalar_tensor_tensor(
                    out=dotscr[:], in0=A2[:, H * pl:H * (pl + 1)], scalar=SCALE,
                    in1=h2[:], op0=ALU.mult, op1=ALU.mult,
                    accum_out=dot_sb[:, pl:pl + 1])
            # softmax via exp(x) = (1+tanh(x/2))/(1-tanh(x/2)); norm folded below
            nc.scalar.activation(th[:], dot_sb[:], ACTF.Tanh, scale=0.5)
            nc.scalar.activation(thp1[:], th[:], ACTF.Copy, bias=1.0)
            nc.scalar.activation(om[:], th[:], ACTF.Copy, bias=1.0, scale=-1.0)
            nc.vector.reciprocal(rin[:], om[:])
            nc.vector.tensor_tensor(wexp[:], thp1[:], rin[:], ALU.mult)
            nc.vector.tensor_reduce(s8[:], wexp[:], axis=AX.X, op=ALU.add)
            nc.vector.tensor_copy(s8hi_c[:], s8[n:128, :])
            nc.vector.tensor_tensor(ssum[:], s8[0:n, :], s8hi_c[:], ALU.add)
            nc.vector.reciprocal(rs[:], ssum[:])

            # -- x-part matmuls for step t+1 keep the PE busy during attention -
            if t + 1 < T:
                ps_next = psum_mm.tile([128, 2 * 512], f32, tag="psmm")
                xtsn = xts_tiles.pop(t + 1)
                for c in [8, 9, 10, 11]:
                    lhsTn = xtsn[:, n * (c - 8):n * (c - 7)]
                    for j in range(4):
                        mm(ps_next, c, j, lhsTn, start=(c == 8), stop=False)
                ps_tiles[t + 1] = ps_next

            # attn[s,h] = (sum_p wexp[s,p]*A[s,p,h]) / ssum[s]
            # pipelined per 128-wide h-chunk: tree-add -> scale -> transpose -> MM
            w_b = wexp[:].rearrange("q (r pl) -> q r pl", r=1).broadcast_to(
                [128, 128, 8])
            for ci in range(4):
                hsl = slice(1024 * ci, 1024 * (ci + 1))
                pA = prodA[:, hsl].rearrange("q (h pl) -> q h pl", pl=8)
                nc.vector.tensor_tensor(
                    pA, A2b[:, hsl].rearrange("q (h pl) -> q h pl", pl=8),
                    w_b, ALU.mult)
                v4 = tr4[:, 512 * ci:512 * (ci + 1)].rearrange(
                    "q (h pl) -> q h pl", pl=4)
                v2 = tr2[:, 256 * ci:256 * (ci + 1)].rearrange(
                    "q (h pl) -> q h pl", pl=2)
                nc.vector.tensor_tensor(v4, pA[:, :, 0:4], pA[:, :, 4:8],
                                        ALU.add)
                nc.vector.tensor_tensor(v2, v4[:, :, 0:2], v4[:, :, 2:4],
                                        ALU.add)
                csl = slice(128 * ci, 128 * (ci + 1))
                nc.vector.tensor_tensor(
                    attn2[:, csl].rearrange("q (h r) -> q h r", r=1),
                    v2[:, :, 0:1], v2[:, :, 1:2], ALU.add)
                nc.vector.tensor_copy(attn2hi_c[:, csl], attn2[n:128, csl])
                nc.vector.tensor_tensor(attn_ps[:, csl], attn2[0:n, csl],
                                        attn2hi_c[:, csl], ALU.add)
                nc.scalar.activation(attn_sm[:, csl], attn_ps[:, csl],
                                     ACTF.Copy, scale=rs[:])
                pt = psum_tp.tile([128, n], bf16)
                nc.tensor.transpose(pt[:], attn_sm[:, csl], id_sb[:])
                nc.scalar.copy(attnT[:, n * ci:n * (ci + 1)], pt[:])
                c = 4 + ci
                lhsT = attnT[:, n * ci:n * (ci + 1)]
                mm(ps, c, 0, lhsT, start=False, stop=(c == 7))
                mm(ps, c, 1, lhsT, start=False, stop=(c == 7))

            # -- gates (pipelined against remaining phase B matmuls) -----------
            nc.scalar.activation(gi[:], ps[0:n, 0:512], ACTF.Sigmoid)
            nc.scalar.activation(gf[:], ps[n:128, 0:512], ACTF.Sigmoid)
            nc.vector.tensor_tensor(t1[:], gf[:], c_st[:], ALU.mult)
            for c in [4, 5, 6, 7]:
                lhsT = attnT[:, n * (c - 4):n * (c - 3)]
                mm(ps, c, 2, lhsT, start=False, stop=(c == 7))
                mm(ps, c, 3, lhsT, start=False, stop=(c == 7))
            nc.scalar.activation(gg[:], ps[n:128, 512:1024], ACTF.Tanh)
            nc.scalar.activation(go[:], ps[0:n, 512:1024], ACTF.Sigmoid)
            nc.vector.tensor_tensor(t2[:], gi[:], gg[:], ALU.mult)
            nc.vector.tensor_tensor(c_st[:], t1[:], t2[:], ALU.add)
            nc.scalar.activation(tct[:], c_st[:], ACTF.Tanh)
            nc.vector.tensor_tensor(h2[0:n, :], go[:], tct[:], ALU.mult)
            nc.vector.tensor_copy(h2[n:128, :], h2[0:n, :])

            nc.gpsimd.dma_start(hs[t], h2[0:n, :])

            for ci in range(4):
                pt = psum_tp.tile([128, n], bf16)
                nc.tensor.transpose(pt[:], h2[0:n, 128 * ci:128 * (ci + 1)],
                                    id_sb[:])
                nc.scalar.copy(hT[:, n * ci:n * (ci + 1)], pt[:])

    nc.compile()
    _cache["nc"] = nc
    return nc


LAST_RESULT = None


def kernel(x, A, Wx, Wh, Wattn, b):
    import os
    import ml_dtypes
    from concourse import bass_utils

    nc = _build_kernel()
    bft = ml_dtypes.bfloat16

    Wcat = np.concatenate([np.asarray(Wh), np.asarray(Wattn), np.asarray(Wx)],
                          axis=0)                         # (1536, 2048)
    Wc_host = np.ascontiguousarray(
        Wcat.reshape(12, 128, H4).transpose(1, 0, 2).reshape(128, 12 * H4)
    ).astype(bft)
    b_host = np.asarray(b, dtype=np.float32).reshape(1, H4).astype(bft)
    ident = np.eye(n, dtype=np.float32).astype(bft)
    ones_h = np.ones((1, n), dtype=bft)

    in_maps = []
    for k in range(NC):
        xc = np.asarray(x[n * k:n * (k + 1)], dtype=np.float32)   # (64, T, D)
        Ac = np.asarray(A[n * k:n * (k + 1)], dtype=np.float32)   # (64, H, 4, 4)
        xh_host = np.ascontiguousarray(
            xc.transpose(1, 2, 0).reshape(T, 4, 128, n)
            .transpose(0, 2, 1, 3).reshape(T, 128, 4 * n)).astype(bft)
        A_flat = Ac.reshape(n, H, 16).transpose(0, 2, 1)          # (n, 16, H)
        A4 = A_flat.reshape(n, 2, 8, H)
        A2_host = np.ascontiguousarray(
            A4.transpose(1, 0, 2, 3).reshape(128, 8 * H)).astype(bft)
        A2b_host = np.ascontiguousarray(
            A4.transpose(1, 0, 3, 2).reshape(128, H * 8)).astype(bft)
        in_maps.append({
            "xh": xh_host,
            "A2d": A2_host,
            "A2bd": A2b_host,
            "Wc": Wc_host,
            "bvec": b_host,
            "identd": ident,
            "onesd": ones_h,
        })

    trace = os.environ.get("KERNEL_TRACE") == "1"
    tmpdir = os.environ.get("KERNEL_TRACE_DIR") or None
    res = bass_utils.run_bass_kernel_spmd(
        nc, in_maps, core_ids=list(range(NC)), trace=trace, tmpdir=tmpdir
    )
    global LAST_RESULT
    LAST_RESULT = res

    out = np.empty((N, T, H), dtype=np.float32)
    for k in range(NC):
        hs_k = np.asarray(res.results[k]["hs"])           # (T, n, H) bf16
        out[n * k:n * (k + 1)] = hs_k.transpose(1, 0, 2).astype(np.float32)
    return out
